# revision 34
# baseline (speedup 1.0000x reference)
"""Trainium2 Bass kernel for nn_CrossAttention_15006615733765 (raw Bass, no Tile).

Mathematical structure: the reference broadcasts a per-batch context vector
(B, CTX_DIM) to every spatial position before projecting to K/V.  All keys
within a batch are therefore identical, softmax over the key axis is exactly
uniform, and the attention output equals V itself.  The module collapses to

    out[b, c, h, w] = ((context[b] @ Wv) @ Wo + bo)[c]

independent of x, Wq and Wk.  The kernel computes the two matmuls on the
tensor engine in bf16 (measured rel err vs the fp32 reference ~4e-3, gate is
2e-2) and materializes the broadcast output shard per core, sharding the 512
output channels across 8 cores.

Timing model (from perfetto traces): the NEFF carries a fixed ~8.2us exit
epilogue (all-engine barrier + a 253-semaphore reset sweep, bounded by the
tensor engine's ~115ns/reset chain) that starts once every engine's program
ends.  The output store's data transfer overlaps that sweep entirely, so
exec time == (time the last engine instruction retires) + ~8.2us.  The
kernel therefore minimizes the program tail, not DMA completion:

  Sync   : ctx+Wo pack, Wv half 0 (HWDGE q), then the broadcast store issue
  Scalar : Wv half 1, sel/id/bias pack (HWDGE q), then Act-engine copies
           (t half A, rep replica 2) to offload the DVE
  Tensor : warmup (clock ramp) -> stage1 (half1 chunks first, pipelined with
           the two half DMAs) -> transposes -> stage2 -> selector matmuls
           (bias folded in via the aux rhs)
  Vector : t half B, tT halves (pipelined with transposes), y, rep 0/1
  GpSimd : unused (no_gpsimd_drain)
"""

import numpy as np
import ml_dtypes

import concourse.bacc as bacc
import concourse.mybir as mybir
from concourse.bass_utils import run_bass_kernel_spmd

B, DIM, CTX_DIM = 4, 512, 768
H = W = 48
NPOS = H * W
NCORES = 8
CPC = DIM // NCORES
P = 128
KC = CTX_DIM // P
KD = DIM // P
ROW = B * CPC
NDUP = 3  # output row replicas per partition -> 3*512B = 1.5KiB descriptors
NWARM = 3  # PE clock-ramp dummy matmuls (0 for CoreSim: reads uninit SBUF)
TRANSPOSE_F32 = True  # fp32 PE transpose path (bf16 PSUM untested on HW)
ACT_COPIES = False  # offload two PSUM->SBUF hops to the Act engine
BF16 = mybir.dt.bfloat16
F32 = mybir.dt.float32
NP_BF16 = ml_dtypes.bfloat16

# wb pack: [:, 0:24] ctx chunks, [:, 24:280] Wo shard chunks
WB_CTX = KC * B          # 24
WB_W = WB_CTX + KD * CPC  # 280
# aux pack: [:, 0:512] selectors, [:, 512:516] transpose identity,
#           [:, 516:580] y rows (0-3, device-written) + bias row (4, host)
AX_ID = B * P            # 512
AX_Y = AX_ID + B         # 516
AX_END = AX_Y + CPC      # 580

_CACHE: dict = {}


def _build_nc():
    nc = bacc.Bacc("TRN2", target_bir_lowering=False, debug=False, num_devices=NCORES)

    tdt = F32 if TRANSPOSE_F32 else BF16

    wvc = nc.dram_tensor("wvc", [P, KC, DIM], BF16, kind="ExternalInput")
    wbc = nc.dram_tensor("wbc", [P, WB_W], BF16, kind="ExternalInput")
    auxc = nc.dram_tensor("auxc", [B + 1, AX_END], BF16, kind="ExternalInput")
    if TRANSPOSE_F32:
        idc = nc.dram_tensor("idc", [B, B], F32, kind="ExternalInput")
    outd = nc.dram_tensor("outd", [NPOS, ROW], BF16, kind="ExternalOutput")

    wv_sb = nc.alloc_sbuf_tensor("wv_sb", [P, KC, DIM], BF16).ap()
    wb_sb = nc.alloc_sbuf_tensor("wb_sb", [P, WB_W], BF16).ap()
    aux_sb = nc.alloc_sbuf_tensor("aux_sb", [B + 1, AX_END], BF16).ap()
    if TRANSPOSE_F32:
        id_sb = nc.alloc_sbuf_tensor("id_sb", [B, B], F32).ap()
    t_sb = nc.alloc_sbuf_tensor("t_sb", [B, DIM], tdt).ap()
    tT_sb = nc.alloc_sbuf_tensor("tT_sb", [P, KD, B], BF16).ap()
    rep_sb = nc.alloc_sbuf_tensor("rep_sb", [P, NDUP, ROW], BF16).ap()

    pt = nc.alloc_psum_tensor("pt", [B, DIM], F32).ap()
    ptT = nc.alloc_psum_tensor("ptT", [P, KD, B], tdt).ap()
    po = nc.alloc_psum_tensor("po", [B, CPC], F32).ap()
    prep = nc.alloc_psum_tensor("prep", [P, B, CPC], F32).ap()
    pwarm = nc.alloc_psum_tensor("pwarm", [B, DIM], F32).ap()

    def ctx_chunk(k):
        return wb_sb[:, B * k:B * (k + 1)]

    def wo_chunk(m):
        return wb_sb[:, WB_CTX + CPC * m:WB_CTX + CPC * (m + 1)]

    prep_flat = prep.rearrange("p b c -> p (b c)")

    from contextlib import ExitStack

    with ExitStack() as stack:
        s_ctx = stack.enter_context(nc.semaphore("s_ctx"))
        s_wv = [stack.enter_context(nc.semaphore(f"s_wv{k}")) for k in range(KC)]
        s_aux = stack.enter_context(nc.semaphore("s_aux"))
        s_s1 = stack.enter_context(nc.semaphore("s_s1"))
        s_tA = stack.enter_context(nc.semaphore("s_tA"))
        s_tB = stack.enter_context(nc.semaphore("s_tB"))
        s_tr = stack.enter_context(nc.semaphore("s_tr"))
        s_tT = stack.enter_context(nc.semaphore("s_tT"))
        s_st2 = stack.enter_context(nc.semaphore("s_st2"))
        s_o5 = stack.enter_context(nc.semaphore("s_o5"))
        s_sel = stack.enter_context(nc.semaphore("s_sel"))
        s_r0 = stack.enter_context(nc.semaphore("s_r0"))
        s_rep = stack.enter_context(nc.semaphore("s_rep"))
        s_out = stack.enter_context(nc.semaphore("s_out"))

        with nc.Block(no_gpsimd_drain=True) as block:

            @block.sync
            def _(sync):
                # ctx first (gates all stage-1), then even wv chunks; odd
                # chunks ride the scalar queue (which has ~1.4us more
                # startup latency), so stage-1 consumes 0,2,4 then 1,3,5.
                sync.dma_start(out=wb_sb[:], in_=wbc[:]).then_inc(s_ctx, 16)
                for k in (0, 2, 4):
                    sync.dma_start(
                        out=wv_sb[:, k, :], in_=wvc[:, k, :]
                    ).then_inc(s_wv[k], 16)
                sync.wait_ge(s_r0, 1)
                sync.wait_ge(s_rep, 2)
                # 1.5KiB contiguous descriptors: chunk (p, r) covers output
                # rows r*384 + p*3 .. +2 (all rows are identical).
                out_view = outd.rearrange("(r p d) n -> p r (d n)", p=P, d=NDUP)
                src_view = (
                    rep_sb.rearrange("p d n -> p (d n)")[:, None, :]
                    .broadcast_to((P, NPOS // (NDUP * P), NDUP * ROW))
                )
                # No explicit completion wait: the block-exit DRAIN on the
                # issuing engine waits for its HWDGE queue to empty, and the
                # data overlaps the fixed semaphore-reset epilogue.
                sync.dma_start(out=out_view, in_=src_view).then_inc(s_out, 16)

            @block.scalar
            def _(scalar):
                for k in (1, 3, 5):
                    scalar.dma_start(
                        out=wv_sb[:, k, :], in_=wvc[:, k, :]
                    ).then_inc(s_wv[k], 16)
                scalar.dma_start(out=aux_sb[:], in_=auxc[:]).then_inc(s_aux, 16)
                if TRANSPOSE_F32:
                    scalar.dma_start(out=id_sb[:], in_=idc[:]).then_inc(s_aux, 16)
                if ACT_COPIES:
                    # Act-engine PSUM->SBUF hops offloading the DVE chain.
                    scalar.wait_ge(s_s1, 1)
                    nc.scalar.copy(
                        t_sb[:, :DIM // 2], pt[:, :DIM // 2]
                    ).then_inc(s_tA, 1)
                    scalar.wait_ge(s_sel, 1)
                    nc.scalar.copy(rep_sb[:, 2, :], prep_flat).then_inc(s_rep, 1)

            @block.tensor
            def _(tensor):
                # Warmup: ungated dummy matmuls (garbage SBUF data, scratch
                # PSUM) keep the PE busy from boot, ramping the PE clock.
                for _w in range(NWARM):
                    nc.tensor.matmul(
                        pwarm[:],
                        wb_sb[:, 0:B],
                        wv_sb[:, KC - 1, :],
                        start=(_w == 0),
                        stop=(_w == NWARM - 1),
                    )
                tensor.wait_ge(s_ctx, 16)
                # stage 1: tmp[b, d] = sum_c ctx[b, c] Wv[c, d], one matmul
                # per single-chunk DMA, consumed in arrival order.
                for k in (0, 2, 4, 1, 3, 5):
                    tensor.wait_ge(s_wv[k], 16)
                    ins = nc.tensor.matmul(
                        pt[:], ctx_chunk(k), wv_sb[:, k, :],
                        start=(k == 0), stop=(k == 5),
                    )
                ins.then_inc(s_s1, 1)

                # transpose tmp -> [d, b] (PE transpose via identity)
                id_ap = id_sb[:] if TRANSPOSE_F32 else aux_sb[0:B, AX_ID:AX_ID + B]
                tensor.wait_ge(s_aux, 32 if TRANSPOSE_F32 else 16)
                tensor.wait_ge(s_tA, 1)
                for m in (0, 1):
                    nc.tensor.transpose(
                        ptT[:, m, :], t_sb[:, m * P:(m + 1) * P], id_ap
                    )
                tensor.wait_ge(s_tB, 1)
                for m in (2, 3):
                    ins = nc.tensor.transpose(
                        ptT[:, m, :], t_sb[:, m * P:(m + 1) * P], id_ap
                    )
                ins.then_inc(s_tr, 1)

                # stage 2: y[b, j] = sum_d tmp[b, d] Wo[d, j]
                tensor.wait_ge(s_tT, 1)
                for m in (0, 1, 2, 3):
                    ins = nc.tensor.matmul(
                        po[:], tT_sb[:, m, :], wo_chunk(m),
                        start=(m == 0), stop=(m == 3),
                    )
                ins.then_inc(s_st2, 1)

                # selector matmuls: prep[p, b, j] = y[b, j] + bo[j] on all
                # 128 partitions (bias row folded into the aux rhs).
                tensor.wait_ge(s_o5, 1)
                for b in range(B):
                    ins = nc.tensor.matmul(
                        prep[:, b, :],
                        aux_sb[:, P * b:P * (b + 1)],
                        aux_sb[:, AX_Y:AX_END],
                        start=True,
                        stop=True,
                    )
                ins.then_inc(s_sel, 1)

            @block.vector
            def _(vector):
                vector.wait_ge(s_s1, 1)
                if not ACT_COPIES:
                    nc.vector.tensor_copy(
                        t_sb[:, :DIM // 2], pt[:, :DIM // 2]
                    ).then_inc(s_tA, 1)
                nc.vector.tensor_copy(
                    t_sb[:, DIM // 2:], pt[:, DIM // 2:]
                ).then_inc(s_tB, 1)
                vector.wait_ge(s_tr, 1)
                nc.vector.tensor_copy(tT_sb[:], ptT[:]).then_inc(s_tT, 1)
                vector.wait_ge(s_st2, 1)
                nc.vector.tensor_copy(
                    aux_sb[0:B, AX_Y:AX_END], po[:]
                ).then_inc(s_o5, 1)
                vector.wait_ge(s_sel, 1)
                nc.vector.tensor_copy(rep_sb[:, 0, :], prep_flat).then_inc(s_r0, 1)
                nc.vector.tensor_copy(rep_sb[:, 1, :], prep_flat).then_inc(s_rep, 1)

            @block.gpsimd
            def _(gpsimd):
                # SBUF->SBUF replica (GpSimd cannot read PSUM, but this can
                # run in parallel with the DVE's second PSUM cast).
                gpsimd.wait_ge(s_r0, 1)
                nc.gpsimd.tensor_copy(
                    rep_sb[:, 2, :], rep_sb[:, 0, :]
                ).then_inc(s_rep, 1)

    nc.compile()
    return nc


def _get_nc():
    if "nc" not in _CACHE:
        _CACHE["nc"] = _build_nc()
    return _CACHE["nc"]


def _bf(a):
    return np.ascontiguousarray(np.asarray(a, dtype=np.float32).astype(NP_BF16))


def _prepare_in_maps(context, Wv, Wo, bo):
    context = np.asarray(context, dtype=np.float32)
    Wv = np.asarray(Wv, dtype=np.float32)
    Wo = np.asarray(Wo, dtype=np.float32)
    bo = np.asarray(bo, dtype=np.float32)

    wvc = _bf(Wv.reshape(KC, P, DIM).transpose(1, 0, 2))
    ctx_pack = context.T.reshape(KC, P, B).transpose(1, 0, 2).reshape(P, KC * B)

    in_maps = []
    for i in range(NCORES):
        wo_shard = Wo[:, i * CPC:(i + 1) * CPC]
        wo_pack = wo_shard.reshape(KD, P, CPC).transpose(1, 0, 2).reshape(P, KD * CPC)
        wbc = _bf(np.concatenate([ctx_pack, wo_pack], axis=1))

        auxc = np.zeros((B + 1, AX_END), dtype=np.float32)
        for b in range(B):
            auxc[b, P * b:P * (b + 1)] = 1.0
            auxc[b, AX_ID + b] = 1.0
        auxc[B, 0:AX_ID] = 1.0
        auxc[B, AX_Y:AX_END] = bo[i * CPC:(i + 1) * CPC]
        m = {"wvc": wvc, "wbc": wbc, "auxc": _bf(auxc)}
        if TRANSPOSE_F32:
            m["idc"] = np.eye(B, dtype=np.float32)
        in_maps.append(m)
    return in_maps


def _unshard(results):
    shards = np.stack(
        [np.asarray(r["outd"]).astype(np.float32) for r in results], axis=0
    )
    shards = shards.reshape(NCORES, NPOS, B, CPC)
    out = shards.transpose(2, 0, 3, 1).reshape(B, DIM, H, W)
    return np.ascontiguousarray(out)


def kernel(x, context, Wq, Wk, Wv, Wo, bo):
    del x, Wq, Wk
    nc = _get_nc()
    in_maps = _prepare_in_maps(context, Wv, Wo, bo)
    results = run_bass_kernel_spmd(nc, in_maps, list(range(NCORES))).results
    return _unshard(results)


# revision 39
# speedup vs baseline: 1.0851x; 1.0851x over previous
"""Trainium2 Bass kernel for nn_CrossAttention_15006615733765 (raw Bass, no Tile).

Mathematical structure: the reference broadcasts a per-batch context vector
(B, CTX_DIM) to every spatial position before projecting to K/V.  All keys
within a batch are therefore identical, softmax over the key axis is exactly
uniform, and the attention output equals V itself.  The module collapses to

    out[b, c, h, w] = ((context[b] @ Wv) @ Wo + bo)[c]

independent of x, Wq and Wk.  The kernel computes the two matmuls on the
tensor engine in bf16 (measured rel err vs the fp32 reference ~4e-3, gate is
2e-2) and materializes the broadcast output shard per core, sharding the 512
output channels across 8 cores.

Timing model (from perfetto traces): the NEFF carries a fixed ~8.2us exit
epilogue (all-engine barrier + a 253-semaphore reset sweep, bounded by the
tensor engine's ~115ns/reset chain) that starts once every engine's program
ends.  The output store's data transfer overlaps that sweep entirely, so
exec time == (time the last engine instruction retires) + ~8.2us.  The
kernel therefore minimizes the program tail, not DMA completion:

  Sync   : ctx+Wo pack, Wv half 0 (HWDGE q), then the broadcast store issue
  Scalar : Wv half 1, sel/id/bias pack (HWDGE q), then Act-engine copies
           (t half A, rep replica 2) to offload the DVE
  Tensor : warmup (clock ramp) -> stage1 (half1 chunks first, pipelined with
           the two half DMAs) -> transposes -> stage2 -> selector matmuls
           (bias folded in via the aux rhs)
  Vector : t half B, tT halves (pipelined with transposes), y, rep 0/1
  GpSimd : unused (no_gpsimd_drain)
"""

import numpy as np
import ml_dtypes

import concourse.bacc as bacc
import concourse.mybir as mybir
from concourse.bass_utils import run_bass_kernel_spmd

B, DIM, CTX_DIM = 4, 512, 768
H = W = 48
NPOS = H * W
NCORES = 8
CPC = DIM // NCORES
P = 128
KC = CTX_DIM // P
KD = DIM // P
ROW = B * CPC
NDUP = 3  # output row replicas per partition -> 3*512B = 1.5KiB descriptors
NWARM = 5  # PE clock-ramp dummy matmuls (0 for CoreSim: reads uninit SBUF)
TRANSPOSE_F32 = True  # fp32 PE transpose path (bf16 PSUM untested on HW)
ACT_COPIES = False  # offload two PSUM->SBUF hops to the Act engine
BF16 = mybir.dt.bfloat16
F32 = mybir.dt.float32
NP_BF16 = ml_dtypes.bfloat16

# wb pack: [:, 0:24] ctx chunks, [:, 24:280] Wo shard chunks
WB_CTX = KC * B          # 24
WB_W = WB_CTX + KD * CPC  # 280
# aux pack: [:, 0:512] selectors, [:, 512:516] transpose identity,
#           [:, 516:580] y rows (0-3, device-written) + bias row (4, host)
AX_ID = B * P            # 512
AX_Y = AX_ID + B         # 516
AX_END = AX_Y + CPC      # 580

_CACHE: dict = {}


def _build_nc():
    nc = bacc.Bacc("TRN2", target_bir_lowering=False, debug=False, num_devices=NCORES)

    tdt = F32 if TRANSPOSE_F32 else BF16

    wvc = nc.dram_tensor("wvc", [P, KC, DIM], BF16, kind="ExternalInput")
    wbc = nc.dram_tensor("wbc", [P, WB_W], BF16, kind="ExternalInput")
    auxc = nc.dram_tensor("auxc", [B + 1, AX_END], BF16, kind="ExternalInput")
    if TRANSPOSE_F32:
        idc = nc.dram_tensor("idc", [B, B], F32, kind="ExternalInput")
    outd = nc.dram_tensor("outd", [NPOS, ROW], BF16, kind="ExternalOutput")

    wv_sb = nc.alloc_sbuf_tensor("wv_sb", [P, KC, DIM], BF16).ap()
    wb_sb = nc.alloc_sbuf_tensor("wb_sb", [P, WB_W], BF16).ap()
    aux_sb = nc.alloc_sbuf_tensor("aux_sb", [B + 1, AX_END], BF16).ap()
    if TRANSPOSE_F32:
        id_sb = nc.alloc_sbuf_tensor("id_sb", [B, B], F32).ap()
    t_sb = nc.alloc_sbuf_tensor("t_sb", [B, DIM], tdt).ap()
    tT_sb = nc.alloc_sbuf_tensor("tT_sb", [P, KD, B], BF16).ap()
    rep_sb = nc.alloc_sbuf_tensor("rep_sb", [P, NDUP, ROW], BF16).ap()

    pt = nc.alloc_psum_tensor("pt", [B, DIM], F32).ap()
    ptT = nc.alloc_psum_tensor("ptT", [P, KD, B], tdt).ap()
    po = nc.alloc_psum_tensor("po", [B, CPC], F32).ap()
    prep = nc.alloc_psum_tensor("prep", [P, B, CPC], F32).ap()
    pwarm = nc.alloc_psum_tensor("pwarm", [B, DIM], F32).ap()

    def ctx_chunk(k):
        return wb_sb[:, B * k:B * (k + 1)]

    def wo_chunk(m):
        return wb_sb[:, WB_CTX + CPC * m:WB_CTX + CPC * (m + 1)]

    prep_flat = prep.rearrange("p b c -> p (b c)")

    from contextlib import ExitStack

    with ExitStack() as stack:
        s_ctx = stack.enter_context(nc.semaphore("s_ctx"))
        s_wv = [stack.enter_context(nc.semaphore(f"s_wv{k}")) for k in range(KC)]
        s_aux = stack.enter_context(nc.semaphore("s_aux"))
        s_s1 = stack.enter_context(nc.semaphore("s_s1"))
        s_tA = stack.enter_context(nc.semaphore("s_tA"))
        s_tB = stack.enter_context(nc.semaphore("s_tB"))
        s_tr = stack.enter_context(nc.semaphore("s_tr"))
        s_tT = stack.enter_context(nc.semaphore("s_tT"))
        s_st2 = stack.enter_context(nc.semaphore("s_st2"))
        s_o5 = stack.enter_context(nc.semaphore("s_o5"))
        s_sel = stack.enter_context(nc.semaphore("s_sel"))
        s_rep = stack.enter_context(nc.semaphore("s_rep"))
        s_out = stack.enter_context(nc.semaphore("s_out"))

        with nc.Block(no_gpsimd_drain=True) as block:

            @block.sync
            def _(sync):
                # ctx first (gates all stage-1), then even wv chunks; odd
                # chunks ride the scalar queue (which has ~1.4us more
                # startup latency), so stage-1 consumes 0,2,4 then 1,3,5.
                sync.dma_start(out=wb_sb[:], in_=wbc[:]).then_inc(s_ctx, 16)
                for k in (0, 2, 4):
                    sync.dma_start(
                        out=wv_sb[:, k, :], in_=wvc[:, k, :]
                    ).then_inc(s_wv[k], 16)
                sync.wait_ge(s_rep, 3)
                # 1.5KiB contiguous descriptors: chunk (p, r) covers output
                # rows r*384 + p*3 .. +2 (all rows are identical).
                out_view = outd.rearrange("(r p d) n -> p r (d n)", p=P, d=NDUP)
                src_view = (
                    rep_sb.rearrange("p d n -> p (d n)")[:, None, :]
                    .broadcast_to((P, NPOS // (NDUP * P), NDUP * ROW))
                )
                # No explicit completion wait: the block-exit DRAIN on the
                # issuing engine waits for its HWDGE queue to empty, and the
                # data overlaps the fixed semaphore-reset epilogue.
                sync.dma_start(out=out_view, in_=src_view).then_inc(s_out, 16)

            @block.scalar
            def _(scalar):
                for k in (1, 3, 5):
                    scalar.dma_start(
                        out=wv_sb[:, k, :], in_=wvc[:, k, :]
                    ).then_inc(s_wv[k], 16)
                scalar.dma_start(out=aux_sb[:], in_=auxc[:]).then_inc(s_aux, 16)
                if TRANSPOSE_F32:
                    scalar.dma_start(out=id_sb[:], in_=idc[:]).then_inc(s_aux, 16)
                if ACT_COPIES:
                    # Act-engine PSUM->SBUF hops offloading the DVE chain.
                    scalar.wait_ge(s_s1, 1)
                    nc.scalar.copy(
                        t_sb[:, :DIM // 2], pt[:, :DIM // 2]
                    ).then_inc(s_tA, 1)
                    scalar.wait_ge(s_sel, 1)
                    nc.scalar.copy(rep_sb[:, 2, :], prep_flat).then_inc(s_rep, 1)

            @block.tensor
            def _(tensor):
                # Warmup: ungated dummy matmuls (garbage SBUF data, scratch
                # PSUM) keep the PE busy from boot, ramping the PE clock.
                for _w in range(NWARM):
                    nc.tensor.matmul(
                        pwarm[:],
                        wb_sb[:, 0:B],
                        wv_sb[:, KC - 1, :],
                        start=(_w == 0),
                        stop=(_w == NWARM - 1),
                    )
                tensor.wait_ge(s_ctx, 16)
                # stage 1: tmp[b, d] = sum_c ctx[b, c] Wv[c, d], one matmul
                # per single-chunk DMA, consumed in arrival order.
                # consumption order matches measured DMA arrival order (the
                # two HWDGE queues interleave, scalar's first chunk lands
                # between sync's wb and first chunk)
                for i, k in enumerate((1, 0, 3, 2, 5, 4)):
                    tensor.wait_ge(s_wv[k], 16)
                    ins = nc.tensor.matmul(
                        pt[:], ctx_chunk(k), wv_sb[:, k, :],
                        start=(i == 0), stop=(i == 5),
                    )
                ins.then_inc(s_s1, 1)

                # transpose tmp -> [d, b] (PE transpose via identity)
                id_ap = id_sb[:] if TRANSPOSE_F32 else aux_sb[0:B, AX_ID:AX_ID + B]
                tensor.wait_ge(s_aux, 32 if TRANSPOSE_F32 else 16)
                tensor.wait_ge(s_tA, 1)
                for m in (0, 1):
                    nc.tensor.transpose(
                        ptT[:, m, :], t_sb[:, m * P:(m + 1) * P], id_ap
                    )
                tensor.wait_ge(s_tB, 1)
                for m in (2, 3):
                    ins = nc.tensor.transpose(
                        ptT[:, m, :], t_sb[:, m * P:(m + 1) * P], id_ap
                    )
                ins.then_inc(s_tr, 1)

                # stage 2: y[b, j] = sum_d tmp[b, d] Wo[d, j]
                tensor.wait_ge(s_tT, 1)
                for m in (0, 1, 2, 3):
                    ins = nc.tensor.matmul(
                        po[:], tT_sb[:, m, :], wo_chunk(m),
                        start=(m == 0), stop=(m == 3),
                    )
                ins.then_inc(s_st2, 1)

                # selector matmuls: prep[p, b, j] = y[b, j] + bo[j] on all
                # 128 partitions (bias row folded into the aux rhs).
                tensor.wait_ge(s_o5, 1)
                for b in range(B):
                    ins = nc.tensor.matmul(
                        prep[:, b, :],
                        aux_sb[:, P * b:P * (b + 1)],
                        aux_sb[:, AX_Y:AX_END],
                        start=True,
                        stop=True,
                    )
                ins.then_inc(s_sel, 1)

            @block.vector
            def _(vector):
                vector.wait_ge(s_s1, 1)
                if not ACT_COPIES:
                    nc.vector.tensor_copy(
                        t_sb[:, :DIM // 2], pt[:, :DIM // 2]
                    ).then_inc(s_tA, 1)
                nc.vector.tensor_copy(
                    t_sb[:, DIM // 2:], pt[:, DIM // 2:]
                ).then_inc(s_tB, 1)
                vector.wait_ge(s_tr, 1)
                nc.vector.tensor_copy(tT_sb[:], ptT[:]).then_inc(s_tT, 1)
                vector.wait_ge(s_st2, 1)
                nc.vector.tensor_copy(
                    aux_sb[0:B, AX_Y:AX_END], po[:]
                ).then_inc(s_o5, 1)
                vector.wait_ge(s_sel, 1)
                nc.vector.tensor_copy(rep_sb[:, 0, :], prep_flat).then_inc(s_rep, 1)
                nc.vector.tensor_copy(rep_sb[:, 1, :], prep_flat).then_inc(s_rep, 1)
                nc.vector.tensor_copy(rep_sb[:, 2, :], prep_flat).then_inc(s_rep, 1)

    nc.compile()
    return nc


def _get_nc():
    if "nc" not in _CACHE:
        _CACHE["nc"] = _build_nc()
    return _CACHE["nc"]


def _bf(a):
    return np.ascontiguousarray(np.asarray(a, dtype=np.float32).astype(NP_BF16))


def _prepare_in_maps(context, Wv, Wo, bo):
    context = np.asarray(context, dtype=np.float32)
    Wv = np.asarray(Wv, dtype=np.float32)
    Wo = np.asarray(Wo, dtype=np.float32)
    bo = np.asarray(bo, dtype=np.float32)

    wvc = _bf(Wv.reshape(KC, P, DIM).transpose(1, 0, 2))
    ctx_pack = context.T.reshape(KC, P, B).transpose(1, 0, 2).reshape(P, KC * B)

    in_maps = []
    for i in range(NCORES):
        wo_shard = Wo[:, i * CPC:(i + 1) * CPC]
        wo_pack = wo_shard.reshape(KD, P, CPC).transpose(1, 0, 2).reshape(P, KD * CPC)
        wbc = _bf(np.concatenate([ctx_pack, wo_pack], axis=1))

        auxc = np.zeros((B + 1, AX_END), dtype=np.float32)
        for b in range(B):
            auxc[b, P * b:P * (b + 1)] = 1.0
            auxc[b, AX_ID + b] = 1.0
        auxc[B, 0:AX_ID] = 1.0
        auxc[B, AX_Y:AX_END] = bo[i * CPC:(i + 1) * CPC]
        m = {"wvc": wvc, "wbc": wbc, "auxc": _bf(auxc)}
        if TRANSPOSE_F32:
            m["idc"] = np.eye(B, dtype=np.float32)
        in_maps.append(m)
    return in_maps


def _unshard(results):
    shards = np.stack(
        [np.asarray(r["outd"]).astype(np.float32) for r in results], axis=0
    )
    shards = shards.reshape(NCORES, NPOS, B, CPC)
    out = shards.transpose(2, 0, 3, 1).reshape(B, DIM, H, W)
    return np.ascontiguousarray(out)


def kernel(x, context, Wq, Wk, Wv, Wo, bo):
    del x, Wq, Wk
    nc = _get_nc()
    in_maps = _prepare_in_maps(context, Wv, Wo, bo)
    results = run_bass_kernel_spmd(nc, in_maps, list(range(NCORES))).results
    return _unshard(results)
